# revision 22
# baseline (speedup 1.0000x reference)
"""AttentionTeacher Trainium2 kernel.

Math (reference):
    q = query @ Wq.T + bq;  k = key @ Wk.T + bk          [B,S,HID]
    per head h (HD=64): scores_h = q_h k_h^T / 8 + mask  [B,NH,S,S]
    probs_h = softmax(scores_h)
    out = (sum_h probs_h) @ V / NH                       [B,S,HID]

Sharding: 8 cores, SPMD, no collectives. Core i handles batch b=i//2 and
query rows [512*(i%2), 512*(i%2+1)). Each core needs the full K/V of its
batch; the K-projection is duplicated across the pair of cores sharing a
batch, which keeps the total matmul FLOPs equal to the head-parallel
sharding while avoiding the all-reduce.

Host-side prep folds all cheap scalar work into the staged operands:
  - transposed layouts ([din, s] / [din, dout]) so every matmul contraction
    dim lands on SBUF partitions without on-device transposes
  - 1/sqrt(HD) folded into Wq/bq, 1/NH and exp(mask) folded into V
  - bf16 casts (PSUM accumulation stays fp32)

Device pipeline per core (emission order = per-engine execution order, so
projection tiles are interleaved with the first two query blocks' heads to
keep every engine streaming from the start):
  PE:  qT/kT projections -> per-head scores (K=64, tile_position row
       pairs) -> PE-transpose of head-summed P -> P^T @ V
  ACT: exp(scores) from PSUM with per-row sums (Z) via accum_out; all
       PSUM evacuations/casts (projection +bias, P^T, output)
  DVE: normalize+head-sum as a pure-fp32 scalar_tensor_tensor chain
       (P += E_h * (1/Z_h)); reciprocals of Z in groups of 4 heads.
       fp32 is deliberate: measured bf16 DVE ops run 3-5x slower than
       fp32 on this backend (bf16 STT ~3.0us vs fp32 ~0.58us per
       [128,1024] tile), which made an earlier bf16 tree the kernel
       bottleneck.
"""

import numpy as np
import ml_dtypes

import concourse.bass as bass
import concourse.tile as tile
from concourse import bacc, mybir
from concourse.bass_utils import run_bass_kernel_spmd

N_CORES = 8
B, S, HID, NH, HD = 4, 1024, 1024, 16, 64
SQ = S // 2          # query rows per core
QB = SQ // 128       # query blocks per core
DT = HID // 128      # dout tiles (2 heads each)
KTI = HID // 128     # contraction (din) tiles
CD = mybir.dt.bfloat16
F32 = mybir.dt.float32
BF16_NP = ml_dtypes.bfloat16

_ts = bass.ts
_mult = mybir.AluOpType.mult
_add = mybir.AluOpType.add
_EXP = mybir.ActivationFunctionType.Exp

_CACHE: dict = {}


def _build_program(reps: int = 1):
    nc = bacc.Bacc(
        "TRN2", target_bir_lowering=False, debug=False, num_devices=N_CORES
    )
    d_qt = nc.dram_tensor("qt_in", [HID, SQ], CD, kind="ExternalInput")
    d_kt = nc.dram_tensor("kt_in", [HID, S], CD, kind="ExternalInput")
    d_wq = nc.dram_tensor("wqt_in", [HID, HID], CD, kind="ExternalInput")
    d_wk = nc.dram_tensor("wkt_in", [HID, HID], CD, kind="ExternalInput")
    d_v = nc.dram_tensor("v_in", [S, HID], CD, kind="ExternalInput")
    d_bq = nc.dram_tensor("bq_in", [128, DT], F32, kind="ExternalInput")
    d_bk = nc.dram_tensor("bk_in", [128, DT], F32, kind="ExternalInput")
    d_id = nc.dram_tensor("ident_in", [128, 128], F32, kind="ExternalInput")
    d_o = nc.dram_tensor("o_out", [SQ, HID], F32, kind="ExternalOutput")

    _ID = mybir.ActivationFunctionType.Identity

    with tile.TileContext(nc) as tc:
        with (
            tc.tile_pool(name="const", bufs=1) as const_pool,
            tc.tile_pool(name="win", bufs=1) as win_pool,
            tc.tile_pool(name="xin", bufs=1) as xin_pool,
            tc.tile_pool(name="proj", bufs=1) as proj_pool,
            tc.tile_pool(name="e", bufs=18) as e_pool,
            tc.tile_pool(name="z", bufs=14) as z_pool,
            tc.tile_pool(name="pt", bufs=2) as pt_pool,
            tc.tile_pool(name="osb", bufs=2) as o_pool,
            tc.tile_pool(name="proj_ps", bufs=2, space="PSUM") as proj_ps,
            tc.tile_pool(name="sc_ps", bufs=2, space="PSUM") as sc_ps,
            tc.tile_pool(name="ptpv_ps", bufs=2, space="PSUM") as ptpv_ps,
        ):
          for _rep in range(reps):
            # DMA order is the arrival order; V and constants used late go last.
            wq, wk, qin, kin, vsb = [], [], [], [], []
            for i in range(KTI):
                t = win_pool.tile([128, HID], CD, tag=f"wq{i}", name=f"wq{i}")
                nc.sync.dma_start(t[:], d_wq.ap()[_ts(i, 128), :])
                wq.append(t)
                t = xin_pool.tile([128, SQ], CD, tag=f"qin{i}", name=f"qin{i}")
                nc.sync.dma_start(t[:], d_qt.ap()[_ts(i, 128), :])
                qin.append(t)
            for i in range(KTI):
                t = win_pool.tile([128, HID], CD, tag=f"wk{i}", name=f"wk{i}")
                nc.sync.dma_start(t[:], d_wk.ap()[_ts(i, 128), :])
                wk.append(t)
                t = xin_pool.tile([128, S], CD, tag=f"kin{i}", name=f"kin{i}")
                nc.sync.dma_start(t[:], d_kt.ap()[_ts(i, 128), :])
                kin.append(t)
            bq_sb = const_pool.tile([128, DT], F32, tag="bq", name="bq_sb")
            nc.sync.dma_start(bq_sb[:], d_bq.ap()[:])
            bk_sb = const_pool.tile([128, DT], F32, tag="bk", name="bk_sb")
            nc.sync.dma_start(bk_sb[:], d_bk.ap()[:])
            for i in range(KTI):
                t = xin_pool.tile([128, HID], CD, tag=f"v{i}", name=f"v{i}")
                nc.sync.dma_start(t[:], d_v.ap()[_ts(i, 128), :])
                vsb.append(t)
            ident = const_pool.tile([128, 128], F32, tag="ident", name="ident")
            nc.sync.dma_start(ident[:], d_id.ap()[:])

            qt = [
                proj_pool.tile([128, SQ], CD, tag=f"qt{t}", name=f"qt{t}")
                for t in range(DT)
            ]
            ktp = [
                proj_pool.tile([128, S], CD, tag=f"kt{t}", name=f"ktp{t}")
                for t in range(DT)
            ]

            # ---- per-qblock attention state ----
            zts = {}     # qb -> [128, NH] f32 row sums
            es = {}      # (qb, h) -> E tile (f32)
            invz = {}    # (qb, grp) -> [128, 4] f32 reciprocals
            chain = {}   # qb -> running normalized head-sum tile (f32)

            def emit_proj(t):
                # PSUM evacuations (+bias, bf16 cast) ride on ScalarE.
                ps = proj_ps.tile([128, SQ], F32, tag="proj", name="proj_q_ps")
                for i in range(KTI):
                    nc.tensor.matmul(
                        ps[:], wq[i][:, _ts(t, 128)], qin[i][:],
                        start=(i == 0), stop=(i == KTI - 1),
                    )
                nc.scalar.activation(
                    qt[t][:], ps[:], _ID, bias=bq_sb[:, t : t + 1]
                )
                for nh in range(2):
                    ps2 = proj_ps.tile([128, 512], F32, tag="proj", name="proj_k_ps")
                    for i in range(KTI):
                        nc.tensor.matmul(
                            ps2[:], wk[i][:, _ts(t, 128)],
                            kin[i][:, _ts(nh, 512)],
                            start=(i == 0), stop=(i == KTI - 1),
                        )
                    nc.scalar.activation(
                        ktp[t][:, _ts(nh, 512)], ps2[:], _ID,
                        bias=bk_sb[:, t : t + 1],
                    )

            def emit_head(qb, h):
                if qb not in zts:
                    zts[qb] = z_pool.tile([128, NH], F32, tag="z", name="zt")
                t, half = h // 2, h % 2
                d0 = 64 * half
                sc = sc_ps.tile([128, S], F32, tag="sc", name="sc")
                for n2 in range(2):
                    nc.tensor.matmul(
                        sc[:, _ts(n2, 512)],
                        qt[t][d0 : d0 + 64, _ts(qb, 128)],
                        ktp[t][d0 : d0 + 64, _ts(n2, 512)],
                        start=True, stop=True, tile_position=(d0, 0),
                    )
                e = e_pool.tile([128, S], F32, tag="e", name="e")
                nc.scalar.activation(e[:], sc[:], _EXP, accum_out=zts[qb][:, h : h + 1])
                es[(qb, h)] = e

            def emit_recip(qb, grp):
                inv = z_pool.tile([128, 4], F32, tag="z", name="inv_z")
                nc.vector.reciprocal(inv[:], zts[qb][:, grp * 4 : grp * 4 + 4])
                invz[(qb, grp)] = inv

            def emit_chain(qb, h):
                # running P += E_h / Z_h  (fp32 scalar_tensor_tensor chain)
                sv = invz[(qb, h // 4)][:, h % 4 : h % 4 + 1]
                nxt = e_pool.tile([128, S], F32, tag="e", name="chain")
                if h == 0:
                    nc.vector.tensor_scalar(
                        out=nxt[:], in0=es[(qb, 0)][:], scalar1=sv,
                        scalar2=None, op0=_mult,
                    )
                else:
                    nc.vector.scalar_tensor_tensor(
                        out=nxt[:], in0=es[(qb, h)][:], scalar=sv,
                        in1=chain[qb][:], op0=_mult, op1=_add,
                    )
                chain[qb] = nxt

            def emit_pace(qb, i):
                if i == 1:
                    emit_recip(qb, 0)
                elif i == 2:
                    for h in range(4):
                        emit_chain(qb, h)
                elif i == 3:
                    emit_recip(qb, 1)
                elif i == 4:
                    for h in range(4, 8):
                        emit_chain(qb, h)
                elif i == 5:
                    emit_recip(qb, 2)
                elif i == 6:
                    for h in range(8, 12):
                        emit_chain(qb, h)
                elif i == 7:
                    emit_recip(qb, 3)
                    for h in range(12, NH):
                        emit_chain(qb, h)

            def emit_out(qb):
                # PE transpose of P (f32), bf16 conversion on ScalarE, P^T @ V
                p = chain[qb]
                ptsb = pt_pool.tile([128, S], CD, tag="pt", name="ptsb")
                for c in range(2):
                    pp = ptpv_ps.tile([128, 512], F32, tag="ptpv", name="pp")
                    for j in range(4):
                        nc.tensor.transpose(
                            pp[:, _ts(j, 128)], p[:, _ts(4 * c + j, 128)], ident[:]
                        )
                    nc.scalar.copy(ptsb[:, _ts(c, 512)], pp[:])
                osb = o_pool.tile([128, HID], F32, tag="osb", name="osb")
                for n2 in range(2):
                    ov = ptpv_ps.tile([128, 512], F32, tag="ptpv", name="ov")
                    for kt_i in range(KTI):
                        nc.tensor.matmul(
                            ov[:], ptsb[:, _ts(kt_i, 128)],
                            vsb[kt_i][:, _ts(n2, 512)],
                            start=(kt_i == 0), stop=(kt_i == KTI - 1),
                        )
                    nc.scalar.copy(osb[:, _ts(n2, 512)], ov[:])
                nc.sync.dma_start(d_o.ap()[_ts(qb, 128), :], osb[:])

            # ---- emission schedule ----
            for t in range(DT):
                emit_proj(t)
                for qb in (0, 1):
                    emit_head(qb, 2 * t)
                    emit_head(qb, 2 * t + 1)
                    emit_pace(qb, t)
            for i in range(8):
                emit_head(2, 2 * i)
                emit_head(2, 2 * i + 1)
                emit_pace(2, i)
                if i == 2:
                    emit_out(0)
                elif i == 5:
                    emit_out(1)
            for i in range(8):
                emit_head(3, 2 * i)
                emit_head(3, 2 * i + 1)
                emit_pace(3, i)
                if i == 2:
                    emit_out(2)
            emit_out(3)

    nc.compile()
    return nc


def _get_program(reps: int = 1):
    key = f"nc{reps}"
    if key not in _CACHE:
        _CACHE[key] = _build_program(reps)
    return _CACHE[key]


class _Runner:
    """Compile-once SPMD executor (mirrors run_bass_via_pjrt's multi-core
    path, but keeps the jitted function so repeat calls skip re-compile)."""

    def __init__(self, nc):
        import jax
        from jax.sharding import Mesh, PartitionSpec
        from jax.experimental.shard_map import shard_map
        from concourse import bass2jax, mybir as mb

        bass2jax.install_neuronx_cc_hook()
        self.jax = jax
        self.nc = nc
        partition_name = (
            nc.partition_id_tensor.name if nc.partition_id_tensor else None
        )
        in_names, out_names, out_avals = [], [], []
        for alloc in nc.m.functions[0].allocations:
            if not isinstance(alloc, mb.MemoryLocationSet):
                continue
            name = alloc.memorylocations[0].name
            if alloc.kind == "ExternalInput":
                if name != partition_name:
                    in_names.append(name)
            elif alloc.kind == "ExternalOutput":
                out_names.append(name)
                out_avals.append(
                    jax.core.ShapedArray(
                        tuple(alloc.tensor_shape), mb.dt.np(alloc.dtype)
                    )
                )
        self.n_params = len(in_names)
        self.out_names = out_names
        self.out_avals = out_avals
        self.zero_outs = [
            np.zeros((N_CORES * a.shape[0], *a.shape[1:]), a.dtype)
            for a in out_avals
        ]
        all_in_names = list(in_names) + list(out_names)
        if partition_name is not None:
            all_in_names.append(partition_name)
        self.in_names = in_names

        def _body(*args):
            operands = list(args)
            if partition_name is not None:
                operands.append(bass2jax.partition_id_tensor())
            outs = bass2jax._bass_exec_p.bind(
                *operands,
                out_avals=tuple(out_avals),
                in_names=tuple(all_in_names),
                out_names=tuple(out_names),
                lowering_input_output_aliases=(),
                sim_require_finite=True,
                sim_require_nnan=True,
                nc=nc,
            )
            return tuple(outs)

        devices = jax.devices()[:N_CORES]
        mesh = Mesh(np.asarray(devices), ("core",))
        n_all = self.n_params + len(out_names)
        self.fn = jax.jit(
            shard_map(
                _body,
                mesh=mesh,
                in_specs=(PartitionSpec("core"),) * n_all,
                out_specs=(PartitionSpec("core"),) * len(out_names),
                check_rep=False,
            ),
            keep_unused=True,
        )

    def stage(self, in_maps):
        """Concatenate per-core inputs along axis 0 (host-side)."""
        concat = [
            np.concatenate([np.asarray(m[n]) for m in in_maps], axis=0)
            for n in self.in_names
        ]
        return concat + self.zero_outs

    def run_staged(self, staged):
        return self.fn(*staged)

    def __call__(self, in_maps):
        out_arrs = self.fn(*self.stage(in_maps))
        return [
            {
                n: np.asarray(out_arrs[i]).reshape(
                    N_CORES, *self.out_avals[i].shape
                )[c]
                for i, n in enumerate(self.out_names)
            }
            for c in range(N_CORES)
        ]


def _get_runner(reps: int = 1):
    key = f"runner{reps}"
    if key not in _CACHE:
        _CACHE[key] = _Runner(_get_program(reps))
    return _CACHE[key]


def make_in_maps(attention_mask, query, key, value, Wq, bq, Wk, bk):
    """Host-side shard + layout prep. Returns per-core input dicts."""
    attention_mask = np.asarray(attention_mask, dtype=np.float32)
    query = np.asarray(query, dtype=np.float32)
    key = np.asarray(key, dtype=np.float32)
    value = np.asarray(value, dtype=np.float32)
    Wq = np.asarray(Wq, dtype=np.float32)
    bq = np.asarray(bq, dtype=np.float32)
    Wk = np.asarray(Wk, dtype=np.float32)
    bk = np.asarray(bk, dtype=np.float32)

    scale = 1.0 / np.sqrt(np.float32(HD))
    wqt = np.ascontiguousarray((Wq * scale).T).astype(BF16_NP)  # [din, dout]
    wkt = np.ascontiguousarray(Wk.T).astype(BF16_NP)
    bq_t = np.ascontiguousarray((bq * scale).reshape(DT, 128).T).astype(np.float32)
    bk_t = np.ascontiguousarray(bk.reshape(DT, 128).T).astype(np.float32)
    ident = np.eye(128, dtype=np.float32)

    in_maps = []
    for core in range(N_CORES):
        b, qh = divmod(core, 2)
        q0 = qh * SQ
        qt_in = np.ascontiguousarray(query[b, q0 : q0 + SQ, :].T).astype(BF16_NP)
        kt_in = np.ascontiguousarray(key[b].T).astype(BF16_NP)
        w = np.exp(attention_mask[b, 0, 0, :]).astype(np.float32) / np.float32(NH)
        v_in = (value[b] * w[:, None]).astype(BF16_NP)
        in_maps.append(
            {
                "qt_in": qt_in,
                "kt_in": kt_in,
                "wqt_in": wqt,
                "wkt_in": wkt,
                "v_in": v_in,
                "bq_in": bq_t,
                "bk_in": bk_t,
                "ident_in": ident,
            }
        )
    return in_maps


def gather_output(results):
    out = np.empty((B, S, HID), dtype=np.float32)
    for core in range(N_CORES):
        b, qh = divmod(core, 2)
        q0 = qh * SQ
        out[b, q0 : q0 + SQ, :] = results[core]["o_out"]
    return out


def kernel(attention_mask, query, key, value, Wq, bq, Wk, bk):
    runner = _get_runner()
    in_maps = make_in_maps(attention_mask, query, key, value, Wq, bq, Wk, bk)
    return gather_output(runner(in_maps))


if __name__ == "__main__":
    rng = np.random.default_rng(0)
    ins = {
        "attention_mask": np.zeros((B, 1, 1, S), np.float32),
        "query": rng.standard_normal((B, S, HID)).astype(np.float32),
        "key": rng.standard_normal((B, S, HID)).astype(np.float32),
        "value": rng.standard_normal((B, S, HID)).astype(np.float32),
        "Wq": (rng.standard_normal((HID, HID)) * 0.02).astype(np.float32),
        "bq": np.zeros(HID, np.float32),
        "Wk": (rng.standard_normal((HID, HID)) * 0.02).astype(np.float32),
        "bk": np.zeros(HID, np.float32),
    }
    out = kernel(**ins)
    print("kernel output:", out.shape, out.dtype)


# revision 24
# speedup vs baseline: 1.8198x; 1.8198x over previous
"""AttentionTeacher Trainium2 kernel.

Math (reference):
    q = query @ Wq.T + bq;  k = key @ Wk.T + bk          [B,S,HID]
    per head h (HD=64): scores_h = q_h k_h^T / 8 + mask  [B,NH,S,S]
    probs_h = softmax(scores_h)
    out = (sum_h probs_h) @ V / NH                       [B,S,HID]

Sharding: 8 cores, SPMD, no collectives. Core i handles batch b=i//2 and
query rows [512*(i%2), 512*(i%2+1)). Each core needs the full K/V of its
batch; the K-projection is duplicated across the pair of cores sharing a
batch, which keeps the total matmul FLOPs equal to the head-parallel
sharding while avoiding the all-reduce.

Host-side prep folds all cheap scalar work into the staged operands:
  - transposed layouts ([din, s] / [din, dout]) so every matmul contraction
    dim lands on SBUF partitions without on-device transposes
  - 1/sqrt(HD) folded into Wq/bq, 1/NH and exp(mask) folded into V
  - bf16 casts (PSUM accumulation stays fp32)

Device pipeline per core (emission order = per-engine execution order, so
projection tiles are interleaved with the first two query blocks' heads to
keep every engine streaming from the start):
  PE:  qT/kT projections -> per-head scores (K=64, tile_position row
       pairs) -> PE-transpose of head-summed P -> P^T @ V
  ACT: exp(scores) from PSUM with per-row sums (Z) via accum_out; all
       PSUM evacuations/casts (projection +bias, P^T, output)
  DVE: normalize+head-sum as a pure-fp32 scalar_tensor_tensor chain
       (P += E_h * (1/Z_h)); reciprocals of Z in groups of 4 heads.
       fp32 is deliberate: measured bf16 DVE ops run 3-5x slower than
       fp32 on this backend (bf16 STT ~3.0us vs fp32 ~0.58us per
       [128,1024] tile), which made an earlier bf16 tree the kernel
       bottleneck.
"""

import numpy as np
import ml_dtypes

import concourse.bass as bass
import concourse.tile as tile
from concourse import bacc, mybir
from concourse.bass_utils import run_bass_kernel_spmd

N_CORES = 8
B, S, HID, NH, HD = 4, 1024, 1024, 16, 64
SQ = S // 2          # query rows per core
QB = SQ // 128       # query blocks per core
DT = HID // 128      # dout tiles (2 heads each)
KTI = HID // 128     # contraction (din) tiles
CD = mybir.dt.bfloat16
F32 = mybir.dt.float32
BF16_NP = ml_dtypes.bfloat16

_ts = bass.ts
_mult = mybir.AluOpType.mult
_add = mybir.AluOpType.add
_EXP = mybir.ActivationFunctionType.Exp

_CACHE: dict = {}


def _build_program(reps: int = 1):
    nc = bacc.Bacc(
        "TRN2", target_bir_lowering=False, debug=False, num_devices=N_CORES
    )
    d_qt = nc.dram_tensor("qt_in", [HID, SQ], CD, kind="ExternalInput")
    d_kt = nc.dram_tensor("kt_in", [HID, S], CD, kind="ExternalInput")
    d_wq = nc.dram_tensor("wqt_in", [HID, HID], CD, kind="ExternalInput")
    d_wk = nc.dram_tensor("wkt_in", [HID, HID], CD, kind="ExternalInput")
    d_v = nc.dram_tensor("v_in", [S, HID], CD, kind="ExternalInput")
    d_bq = nc.dram_tensor("bq_in", [128, DT], F32, kind="ExternalInput")
    d_bk = nc.dram_tensor("bk_in", [128, DT], F32, kind="ExternalInput")
    d_id = nc.dram_tensor("ident_in", [128, 128], F32, kind="ExternalInput")
    d_o = nc.dram_tensor("o_out", [SQ, HID], F32, kind="ExternalOutput")

    _ID = mybir.ActivationFunctionType.Identity

    with tile.TileContext(nc) as tc:
        with (
            tc.tile_pool(name="const", bufs=1) as const_pool,
            tc.tile_pool(name="win", bufs=1) as win_pool,
            tc.tile_pool(name="xin", bufs=1) as xin_pool,
            tc.tile_pool(name="proj", bufs=1) as proj_pool,
            tc.tile_pool(name="e", bufs=18) as e_pool,
            tc.tile_pool(name="z", bufs=14) as z_pool,
            tc.tile_pool(name="pt", bufs=2) as pt_pool,
            tc.tile_pool(name="osb", bufs=2) as o_pool,
            tc.tile_pool(name="proj_ps", bufs=2, space="PSUM") as proj_ps,
            tc.tile_pool(name="sc_ps", bufs=2, space="PSUM") as sc_ps,
            tc.tile_pool(name="ptpv_ps", bufs=2, space="PSUM") as ptpv_ps,
        ):
          for _rep in range(reps):
            # DMA order is the arrival order; V and constants used late go last.
            wq, wk, qin, kin, vsb = [], [], [], [], []
            for i in range(KTI):
                t = win_pool.tile([128, HID], CD, tag=f"wq{i}", name=f"wq{i}")
                nc.sync.dma_start(t[:], d_wq.ap()[_ts(i, 128), :])
                wq.append(t)
                t = xin_pool.tile([128, SQ], CD, tag=f"qin{i}", name=f"qin{i}")
                nc.sync.dma_start(t[:], d_qt.ap()[_ts(i, 128), :])
                qin.append(t)
            for i in range(KTI):
                t = win_pool.tile([128, HID], CD, tag=f"wk{i}", name=f"wk{i}")
                nc.sync.dma_start(t[:], d_wk.ap()[_ts(i, 128), :])
                wk.append(t)
                t = xin_pool.tile([128, S], CD, tag=f"kin{i}", name=f"kin{i}")
                nc.sync.dma_start(t[:], d_kt.ap()[_ts(i, 128), :])
                kin.append(t)
            bq_sb = const_pool.tile([128, DT], F32, tag="bq", name="bq_sb")
            nc.sync.dma_start(bq_sb[:], d_bq.ap()[:])
            bk_sb = const_pool.tile([128, DT], F32, tag="bk", name="bk_sb")
            nc.sync.dma_start(bk_sb[:], d_bk.ap()[:])
            for i in range(KTI):
                t = xin_pool.tile([128, HID], CD, tag=f"v{i}", name=f"v{i}")
                nc.sync.dma_start(t[:], d_v.ap()[_ts(i, 128), :])
                vsb.append(t)
            ident = const_pool.tile([128, 128], F32, tag="ident", name="ident")
            nc.sync.dma_start(ident[:], d_id.ap()[:])

            qt = [
                proj_pool.tile([128, SQ], CD, tag=f"qt{t}", name=f"qt{t}")
                for t in range(DT)
            ]
            ktp = [
                proj_pool.tile([128, S], CD, tag=f"kt{t}", name=f"ktp{t}")
                for t in range(DT)
            ]

            # ---- per-qblock attention state ----
            zts = {}     # qb -> [128, NH] f32 row sums
            es = {}      # (qb, h) -> E tile (f32)
            invz = {}    # (qb, grp) -> [128, 4] f32 reciprocals
            chain = {}   # qb -> running normalized head-sum tile (f32)

            def emit_proj(t):
                # PSUM evacuations (+bias, bf16 cast) ride on ScalarE.
                ps = proj_ps.tile([128, SQ], F32, tag="proj", name="proj_q_ps")
                for i in range(KTI):
                    nc.tensor.matmul(
                        ps[:], wq[i][:, _ts(t, 128)], qin[i][:],
                        start=(i == 0), stop=(i == KTI - 1),
                    )
                nc.scalar.activation(
                    qt[t][:], ps[:], _ID, bias=bq_sb[:, t : t + 1]
                )
                for nh in range(2):
                    ps2 = proj_ps.tile([128, 512], F32, tag="proj", name="proj_k_ps")
                    for i in range(KTI):
                        nc.tensor.matmul(
                            ps2[:], wk[i][:, _ts(t, 128)],
                            kin[i][:, _ts(nh, 512)],
                            start=(i == 0), stop=(i == KTI - 1),
                        )
                    nc.scalar.activation(
                        ktp[t][:, _ts(nh, 512)], ps2[:], _ID,
                        bias=bk_sb[:, t : t + 1],
                    )

            def emit_head(qb, h):
                if qb not in zts:
                    zts[qb] = z_pool.tile([128, NH], F32, tag="z", name="zt")
                t, half = h // 2, h % 2
                d0 = 64 * half
                sc = sc_ps.tile([128, S], F32, tag="sc", name="sc")
                for n2 in range(2):
                    nc.tensor.matmul(
                        sc[:, _ts(n2, 512)],
                        qt[t][d0 : d0 + 64, _ts(qb, 128)],
                        ktp[t][d0 : d0 + 64, _ts(n2, 512)],
                        start=True, stop=True, tile_position=(d0, 0),
                    )
                e = e_pool.tile([128, S], F32, tag="e", name="e")
                nc.scalar.activation(e[:], sc[:], _EXP, accum_out=zts[qb][:, h : h + 1])
                es[(qb, h)] = e

            def emit_recip(qb, grp):
                inv = z_pool.tile([128, 4], F32, tag="z", name="inv_z")
                nc.vector.reciprocal(inv[:], zts[qb][:, grp * 4 : grp * 4 + 4])
                invz[(qb, grp)] = inv

            def emit_chain(qb, h):
                # running P += E_h / Z_h  (fp32 scalar_tensor_tensor chain)
                sv = invz[(qb, h // 4)][:, h % 4 : h % 4 + 1]
                nxt = e_pool.tile([128, S], F32, tag="e", name="chain")
                if h == 0:
                    nc.vector.tensor_scalar(
                        out=nxt[:], in0=es[(qb, 0)][:], scalar1=sv,
                        scalar2=None, op0=_mult,
                    )
                else:
                    nc.vector.scalar_tensor_tensor(
                        out=nxt[:], in0=es[(qb, h)][:], scalar=sv,
                        in1=chain[qb][:], op0=_mult, op1=_add,
                    )
                chain[qb] = nxt

            def emit_pace(qb, i):
                if i == 1:
                    emit_recip(qb, 0)
                elif i == 2:
                    for h in range(4):
                        emit_chain(qb, h)
                elif i == 3:
                    emit_recip(qb, 1)
                elif i == 4:
                    for h in range(4, 8):
                        emit_chain(qb, h)
                elif i == 5:
                    emit_recip(qb, 2)
                elif i == 6:
                    for h in range(8, 12):
                        emit_chain(qb, h)
                elif i == 7:
                    emit_recip(qb, 3)
                    for h in range(12, NH):
                        emit_chain(qb, h)

            def emit_out(qb):
                # PE transpose of P (f32), bf16 conversion on ScalarE, P^T @ V
                p = chain[qb]
                ptsb = pt_pool.tile([128, S], CD, tag="pt", name="ptsb")
                for c in range(2):
                    pp = ptpv_ps.tile([128, 512], F32, tag="ptpv", name="pp")
                    for j in range(4):
                        nc.tensor.transpose(
                            pp[:, _ts(j, 128)], p[:, _ts(4 * c + j, 128)], ident[:]
                        )
                    nc.scalar.copy(ptsb[:, _ts(c, 512)], pp[:])
                osb = o_pool.tile([128, HID], F32, tag="osb", name="osb")
                for n2 in range(2):
                    ov = ptpv_ps.tile([128, 512], F32, tag="ptpv", name="ov")
                    for kt_i in range(KTI):
                        nc.tensor.matmul(
                            ov[:], ptsb[:, _ts(kt_i, 128)],
                            vsb[kt_i][:, _ts(n2, 512)],
                            start=(kt_i == 0), stop=(kt_i == KTI - 1),
                        )
                    nc.vector.tensor_copy(osb[:, _ts(n2, 512)], ov[:])
                nc.sync.dma_start(d_o.ap()[_ts(qb, 128), :], osb[:])

            # ---- emission schedule ----
            for t in range(DT):
                emit_proj(t)
                for qb in (0, 1):
                    emit_head(qb, 2 * t)
                    emit_head(qb, 2 * t + 1)
                    emit_pace(qb, t)
            for i in range(8):
                emit_head(2, 2 * i)
                emit_head(2, 2 * i + 1)
                emit_pace(2, i)
                if i == 2:
                    emit_out(0)
                elif i == 5:
                    emit_out(1)
            for i in range(8):
                emit_head(3, 2 * i)
                emit_head(3, 2 * i + 1)
                emit_pace(3, i)
                if i == 2:
                    emit_out(2)
            emit_out(3)

    nc.compile()
    return nc


def _get_program(reps: int = 1):
    key = f"nc{reps}"
    if key not in _CACHE:
        _CACHE[key] = _build_program(reps)
    return _CACHE[key]


class _Runner:
    """Compile-once SPMD executor (mirrors run_bass_via_pjrt's multi-core
    path, but keeps the jitted function so repeat calls skip re-compile)."""

    def __init__(self, nc):
        import jax
        from jax.sharding import Mesh, PartitionSpec
        from jax.experimental.shard_map import shard_map
        from concourse import bass2jax, mybir as mb

        bass2jax.install_neuronx_cc_hook()
        self.jax = jax
        self.nc = nc
        partition_name = (
            nc.partition_id_tensor.name if nc.partition_id_tensor else None
        )
        in_names, out_names, out_avals = [], [], []
        for alloc in nc.m.functions[0].allocations:
            if not isinstance(alloc, mb.MemoryLocationSet):
                continue
            name = alloc.memorylocations[0].name
            if alloc.kind == "ExternalInput":
                if name != partition_name:
                    in_names.append(name)
            elif alloc.kind == "ExternalOutput":
                out_names.append(name)
                out_avals.append(
                    jax.core.ShapedArray(
                        tuple(alloc.tensor_shape), mb.dt.np(alloc.dtype)
                    )
                )
        self.n_params = len(in_names)
        self.out_names = out_names
        self.out_avals = out_avals
        self.zero_outs = [
            np.zeros((N_CORES * a.shape[0], *a.shape[1:]), a.dtype)
            for a in out_avals
        ]
        all_in_names = list(in_names) + list(out_names)
        if partition_name is not None:
            all_in_names.append(partition_name)
        self.in_names = in_names

        def _body(*args):
            operands = list(args)
            if partition_name is not None:
                operands.append(bass2jax.partition_id_tensor())
            outs = bass2jax._bass_exec_p.bind(
                *operands,
                out_avals=tuple(out_avals),
                in_names=tuple(all_in_names),
                out_names=tuple(out_names),
                lowering_input_output_aliases=(),
                sim_require_finite=True,
                sim_require_nnan=True,
                nc=nc,
            )
            return tuple(outs)

        devices = jax.devices()[:N_CORES]
        mesh = Mesh(np.asarray(devices), ("core",))
        n_all = self.n_params + len(out_names)
        self.fn = jax.jit(
            shard_map(
                _body,
                mesh=mesh,
                in_specs=(PartitionSpec("core"),) * n_all,
                out_specs=(PartitionSpec("core"),) * len(out_names),
                check_rep=False,
            ),
            keep_unused=True,
        )

    def stage(self, in_maps):
        """Concatenate per-core inputs along axis 0 (host-side)."""
        concat = [
            np.concatenate([np.asarray(m[n]) for m in in_maps], axis=0)
            for n in self.in_names
        ]
        return concat + self.zero_outs

    def run_staged(self, staged):
        return self.fn(*staged)

    def __call__(self, in_maps):
        out_arrs = self.fn(*self.stage(in_maps))
        return [
            {
                n: np.asarray(out_arrs[i]).reshape(
                    N_CORES, *self.out_avals[i].shape
                )[c]
                for i, n in enumerate(self.out_names)
            }
            for c in range(N_CORES)
        ]


def _get_runner(reps: int = 1):
    key = f"runner{reps}"
    if key not in _CACHE:
        _CACHE[key] = _Runner(_get_program(reps))
    return _CACHE[key]


def make_in_maps(attention_mask, query, key, value, Wq, bq, Wk, bk):
    """Host-side shard + layout prep. Returns per-core input dicts."""
    attention_mask = np.asarray(attention_mask, dtype=np.float32)
    query = np.asarray(query, dtype=np.float32)
    key = np.asarray(key, dtype=np.float32)
    value = np.asarray(value, dtype=np.float32)
    Wq = np.asarray(Wq, dtype=np.float32)
    bq = np.asarray(bq, dtype=np.float32)
    Wk = np.asarray(Wk, dtype=np.float32)
    bk = np.asarray(bk, dtype=np.float32)

    scale = 1.0 / np.sqrt(np.float32(HD))
    wqt = np.ascontiguousarray((Wq * scale).T).astype(BF16_NP)  # [din, dout]
    wkt = np.ascontiguousarray(Wk.T).astype(BF16_NP)
    bq_t = np.ascontiguousarray((bq * scale).reshape(DT, 128).T).astype(np.float32)
    bk_t = np.ascontiguousarray(bk.reshape(DT, 128).T).astype(np.float32)
    ident = np.eye(128, dtype=np.float32)

    in_maps = []
    for core in range(N_CORES):
        b, qh = divmod(core, 2)
        q0 = qh * SQ
        qt_in = np.ascontiguousarray(query[b, q0 : q0 + SQ, :].T).astype(BF16_NP)
        kt_in = np.ascontiguousarray(key[b].T).astype(BF16_NP)
        w = np.exp(attention_mask[b, 0, 0, :]).astype(np.float32) / np.float32(NH)
        v_in = (value[b] * w[:, None]).astype(BF16_NP)
        in_maps.append(
            {
                "qt_in": qt_in,
                "kt_in": kt_in,
                "wqt_in": wqt,
                "wkt_in": wkt,
                "v_in": v_in,
                "bq_in": bq_t,
                "bk_in": bk_t,
                "ident_in": ident,
            }
        )
    return in_maps


def gather_output(results):
    out = np.empty((B, S, HID), dtype=np.float32)
    for core in range(N_CORES):
        b, qh = divmod(core, 2)
        q0 = qh * SQ
        out[b, q0 : q0 + SQ, :] = results[core]["o_out"]
    return out


def kernel(attention_mask, query, key, value, Wq, bq, Wk, bk):
    runner = _get_runner()
    in_maps = make_in_maps(attention_mask, query, key, value, Wq, bq, Wk, bk)
    return gather_output(runner(in_maps))


if __name__ == "__main__":
    rng = np.random.default_rng(0)
    ins = {
        "attention_mask": np.zeros((B, 1, 1, S), np.float32),
        "query": rng.standard_normal((B, S, HID)).astype(np.float32),
        "key": rng.standard_normal((B, S, HID)).astype(np.float32),
        "value": rng.standard_normal((B, S, HID)).astype(np.float32),
        "Wq": (rng.standard_normal((HID, HID)) * 0.02).astype(np.float32),
        "bq": np.zeros(HID, np.float32),
        "Wk": (rng.standard_normal((HID, HID)) * 0.02).astype(np.float32),
        "bk": np.zeros(HID, np.float32),
    }
    out = kernel(**ins)
    print("kernel output:", out.shape, out.dtype)
